# revision 1
# baseline (speedup 1.0000x reference)
"""Multi-head causal attention with RoPE on 8 TRN2 NeuronCores.

Sharding: 2 heads per core (head-parallel QKV + attention), then two
head-split AllToAlls regroup the context to t-sharded cores for the
output projection. All matmul operands are bf16 (1 cycle/row on the PE
at any tile width; PSUM accumulation stays fp32), which also halves DMA
and SBUF footprint vs fp32r.

Layouts (per core, heads hg = 2i, 2i+1):
  qd/kd  SBUF [128=d, 2*4096] bf16  head hl at cols [hl*4096 + t]
                                    partition quadrants of 32 hold rope
                                    pairs as [16 evens; 16 odds] so the
                                    RoPE partner swap is a DVE
                                    stream_shuffle (host permutes Wq/Wk
                                    columns to match)
  vs     SBUF [128=t%128, 32*256]   t-block tbg at cols [tbg*256+(hl*128+dv)]
  scores S^T  PSUM [t=128, r<=512] -> exp -> P^T bf16 in SBUF
  ctx^T  PSUM [dv=128, r=512]       accumulated over t-blocks; denom via
                                    ones-matmul (all rows equal)
  causal: 128-granular — diagonal 512-super streams only cols [j*128, 512)
  out^T  [oc=2048, my 512 t]        host concatenates + transposes

Output projection: Wo (bf16, 8MB) is preloaded in one DMA on the
Activation HWDGE queue at kernel start. After the first A2A (head slot
0 = even heads), partial sums over even g are computed and evicted to
SBUF; after the second A2A the odd-g half accumulates and the halves
are summed on the DVE — so the second A2A's latency hides under PE work.
"""
import sys

if '/opt/trn_rl_repo' not in sys.path:
    sys.path.insert(0, '/opt/trn_rl_repo')

import numpy as np
import ml_dtypes
import concourse.bass as bass  # noqa: F401  (registers bass types)
import concourse.bacc as bacc
import concourse.mybir as mybir
import concourse.tile as tile
from concourse import bass_utils

B, T, D, H, DH = 2, 2048, 2048, 16, 128
NCORES = 8
HPC = H // NCORES          # heads per core = 2
DC = HPC * DH              # output cols per core for q/k/v = 256
BT = B * T                 # 4096
TS = 512                   # t-super / r-super tile
NTS = BT // TS             # 8
KC = D // 128              # 16 contraction chunks
NRS = T // TS              # 4 r-supers per (b, h) pair
SCALE = 1.0 / float(np.sqrt(DH))
ROPE_THETA = 10000.0

_cache = {}


def build(dbg=False, sim=False, reps=1):
    key = ('nc', dbg, sim, reps)
    if key in _cache:
        return _cache[key]
    dtb = mybir.dt.bfloat16
    dtf = mybir.dt.float32
    nc = bacc.Bacc("TRN2", target_bir_lowering=False, debug=False,
                   num_devices=1 if sim else NCORES)

    xT = nc.dram_tensor("xT", [D, BT], dtb, kind="ExternalInput").ap()
    wq = nc.dram_tensor("wq", [D, DC], dtb, kind="ExternalInput").ap()
    wk = nc.dram_tensor("wk", [D, DC], dtb, kind="ExternalInput").ap()
    wv = nc.dram_tensor("wv", [D, DC], dtb, kind="ExternalInput").ap()
    wo = nc.dram_tensor("wo", [D, D], dtb, kind="ExternalInput").ap()
    cosd = nc.dram_tensor("cosd", [128, T], dtb, kind="ExternalInput").ap()
    sind = nc.dram_tensor("sind", [128, T], dtb, kind="ExternalInput").ap()
    mskd = nc.dram_tensor("mskd", [128, 128], dtb, kind="ExternalInput").ap()
    onesd = nc.dram_tensor("onesd", [128, 128], dtb, kind="ExternalInput").ap()
    outp = nc.dram_tensor("out", [D, TS], dtf, kind="ExternalOutput").ap()
    SWAP16 = [(i + 16) % 32 for i in range(32)]  # rope partner swap mask

    with tile.TileContext(nc) as tc:
        with tc.tile_pool(name="const", bufs=1) as constp, \
             tc.tile_pool(name="big", bufs=1) as bigp, \
             tc.tile_pool(name="xt", bufs=7) as xtp, \
             tc.tile_pool(name="rt", bufs=3) as rtp, \
             tc.tile_pool(name="pt", bufs=5) as ptp, \
             tc.tile_pool(name="dv", bufs=2) as dvp, \
             tc.tile_pool(name="cc", bufs=8) as ccp, \
             tc.tile_pool(name="po", bufs=16) as pop, \
             tc.tile_pool(name="ot", bufs=3) as otp, \
             tc.tile_pool(name="ps", bufs=4, space="PSUM") as psp, \
             tc.tile_pool(name="ps2", bufs=2, space="PSUM") as ps2p, \
             tc.tile_pool(name="dram", bufs=1, space="DRAM") as dramp:

            # ---- constants -> SBUF. wq/wk/wv split in 4 chunks: chunk 0 up
            # front, the rest interleaved into ts=0's k-loop so the first
            # matmuls start as early as possible. wo preloaded in 16
            # row-block chunks on the Activation HWDGE queue, two per
            # phase-B ts iteration, so no single transfer hogs the DMA
            # engines. ----
            wq_s = constp.tile([128, KC * DC], dtb)
            wk_s = constp.tile([128, KC * DC], dtb)
            wv_s = constp.tile([128, KC * DC], dtb)
            KQ = KC // 4

            def load_one(dst, src, ks, ke):
                nc.sync.dma_start(
                    dst[:, ks * DC:ke * DC].rearrange(
                        "p (k m) -> p k m", k=ke - ks),
                    src.rearrange("(k p) m -> p k m", p=128)[:, ks:ke])

            def load_w_chunk(c):
                for dst, src in ((wq_s, wq), (wk_s, wk), (wv_s, wv)):
                    load_one(dst, src, c * KQ, (c + 1) * KQ)

            cos_s = constp.tile([128, T], dtb)
            sin_s = constp.tile([128, T], dtb)
            msk_s = constp.tile([128, 128], dtb)
            ones_s = constp.tile([128, 128], dtb)
            wo_s = constp.tile([128, KC * D], dtb)

            # split by batch so phase C's batch-0 work doesn't depend on the
            # still-being-written batch-1 halves
            qd = [bigp.tile([128, HPC * T], dtb, name=f"qd{_b}") for _b in range(B)]
            kd = [bigp.tile([128, HPC * T], dtb, name=f"kd{_b}") for _b in range(B)]
            vs = [bigp.tile([128, (T // 128) * DC], dtb, name=f"vs{_b}")
                  for _b in range(B)]

            send1 = dramp.tile([NCORES * 128, TS], dtb)
            recv1 = dramp.tile([NCORES * 128, TS], dtb)
            send2 = dramp.tile([NCORES * 128, TS], dtb)
            recv2 = dramp.tile([NCORES * 128, TS], dtb)

            def body():
                load_one(wq_s, wq, 0, KQ)
                load_one(wk_s, wk, 0, KQ)
                # defer wv chunk 0 behind xt0: the k=0 V matmuls run after
                # four Q/K matmuls, so xt0 is the more urgent transfer
                with tc.tile_wait_until(0.003):
                    load_one(wv_s, wv, 0, KQ)
                # ---- phase B: projections + RoPE. Q/K accumulate 512-wide; each
                # V t-block accumulator owns its own PSUM tile (interleaved
                # accumulation groups inside one bank corrupt each other). RoPE
                # reads a single SBUF copy of the PSUM result so the Q/K
                # accumulators free as soon as that copy retires. ----
                # (body repeated `reps` times for slope-based HW timing)
                for ts in range(NTS):
                    bb = ts // NRS             # batch this t-super belongs to
                    pos0 = (ts % NRS) * TS     # position within batch
                    psq = [psp.tile([128, TS], dtf, tag="mm", name=f"psq{_h}")
                           for _h in range(2)]
                    psk = [psp.tile([128, TS], dtf, tag="mm", name=f"psk{_h}")
                           for _h in range(2)]
                    psv = [ps2p.tile([128, DC], dtf, tag=("acc" if _t < 2 else "rot"),
                                     name=f"psv{_t}") for _t in range(4)]
                    for k in range(KC):
                        xt = xtp.tile([128, TS], dtb)
                        nc.sync.dma_start(
                            xt[:], xT[k * 128:(k + 1) * 128, ts * TS:(ts + 1) * TS])
                        if ts == 0:
                            if k in (0, 4, 8):
                                load_w_chunk(k // 4 + 1)
                            elif k == 1:
                                # defer non-critical loads past the startup
                                # DMA crunch (wait_until is a scheduler
                                # ordering hint, ms units)
                                with tc.tile_wait_until(0.016):
                                    nc.scalar.dma_start(cos_s[:], cosd[:, :])
                                    nc.scalar.dma_start(sin_s[:], sind[:, :])
                            elif k == 2:
                                with tc.tile_wait_until(0.080):
                                    nc.scalar.dma_start(msk_s[:], mskd[:, :])
                                    nc.scalar.dma_start(ones_s[:], onesd[:, :])
                        if k in (5, 11):       # wo chunks interleaved mid-stream
                            c = 2 * ts + (0 if k == 5 else 1)
                            with tc.tile_wait_until(0.030 + 0.011 * c):
                                nc.scalar.dma_start(wo_s[:, c * D:(c + 1) * D],
                                                    wo[c * 128:(c + 1) * 128, :])
                        st, sp = (k == 0), (k == KC - 1)
                        for hl in range(2):
                            wq_c = wq_s[:, k * DC + hl * 128: k * DC + (hl + 1) * 128]
                            wk_c = wk_s[:, k * DC + hl * 128: k * DC + (hl + 1) * 128]
                            nc.tensor.matmul(psq[hl][:], wq_c, xt[:], start=st, stop=sp)
                            nc.tensor.matmul(psk[hl][:], wk_c, xt[:], start=st, stop=sp)
                        for tb in range(4):
                            nc.tensor.matmul(
                                psv[tb][:], xt[:, tb * 128:(tb + 1) * 128],
                                wv_s[:, k * DC:(k + 1) * DC], start=st, stop=sp)
                    # V eviction on the DVE (scalar stays free for the rope
                    # copies that gate the next ts's Q/K accumulators)
                    for tb in range(4):
                        tbl = (ts % NRS) * 4 + tb
                        nc.vector.tensor_scalar_mul(
                            vs[bb][:, tbl * DC:(tbl + 1) * DC], psv[tb][:], 1.0)
                    # RoPE: o = tmp*cos + shuffle(tmp)*[-sin;sin]; the scalar
                    # copy is the PSUM accumulator's only reader so it frees
                    # fast; the partner swap is a quadrant-local stream_shuffle.
                    for psl, dst in ((psq, qd), (psk, kd)):
                        for hl in range(2):
                            tmp = rtp.tile([128, TS], dtb, tag="tmp")
                            nc.scalar.copy(tmp[:], psl[hl][:])
                            sh = rtp.tile([128, TS], dtb, tag="sh")
                            nc.vector.stream_shuffle(sh[:], tmp[:], SWAP16)
                            odst = dst[bb][:, hl * T + pos0: hl * T + pos0 + TS]
                            nc.vector.tensor_mul(odst, tmp[:],
                                                 cos_s[:, pos0:pos0 + TS])
                            nc.vector.tensor_mul(sh[:], sh[:],
                                                 sin_s[:, pos0:pos0 + TS])
                            nc.vector.tensor_add(odst, odst, sh[:])

                # ---- phase C: attention, pair order (b0,h0),(b1,h0) | (b0,h1),(b1,h1)
                # Diagonal 512-super is 128-granular: block j streams cols
                # [j*128, 512) and only its own j-th 128-col chunk needs masking.
                for hl in range(2):
                    for b in range(B):
                        qh0 = hl * T
                        for R in range(NRS):
                            ps_ctx = ps2p.tile([128, TS], dtf, tag="acc")
                            ps_den = ps2p.tile([128, TS], dtf, tag="rot")
                            ntb = 4 * (R + 1)
                            for tb in range(ntb):
                                j = tb - 4 * R
                                c0 = max(j, 0) * 128     # first valid col
                                ps_s = psp.tile([128, TS], dtf, tag="mm")
                                nc.tensor.matmul(
                                    ps_s[:, c0:TS],
                                    kd[b][:, qh0 + tb * 128: qh0 + (tb + 1) * 128],
                                    qd[b][:, qh0 + R * TS + c0: qh0 + (R + 1) * TS],
                                    start=True, stop=True)
                                pt = ptp.tile([128, TS], dtb)
                                nc.scalar.activation(
                                    pt[:, c0:TS], ps_s[:, c0:TS],
                                    mybir.ActivationFunctionType.Exp, scale=SCALE)
                                if j >= 0:
                                    # mask on the (otherwise idle) Pool engine
                                    # so it never queues behind DVE rope work
                                    nc.gpsimd.tensor_mul(
                                        pt[:, c0:c0 + 128], pt[:, c0:c0 + 128],
                                        msk_s[:, :])
                                vh = vs[b][:, tb * DC + hl * 128: tb * DC + (hl + 1) * 128]
                                st, sp = (tb == 0), (tb == ntb - 1)
                                nc.tensor.matmul(ps_ctx[:, c0:TS], vh, pt[:, c0:TS],
                                                 start=st, stop=sp)
                                nc.tensor.matmul(ps_den[:, c0:TS], ones_s[:],
                                                 pt[:, c0:TS], start=st, stop=sp)
                            rc = dvp.tile([128, TS], dtf)
                            nc.vector.reciprocal(rc[:], ps_den[:])
                            cx = dvp.tile([128, TS], dtb)
                            nc.vector.tensor_mul(cx[:], ps_ctx[:], rc[:])
                            jblk = b * NRS + R
                            sendb = send1 if hl == 0 else send2
                            nc.sync.dma_start(
                                sendb[jblk * 128:(jblk + 1) * 128, :], cx[:])
                    # A2A for this head-slot
                    sendb, recvb = (send1, recv1) if hl == 0 else (send2, recv2)
                    if sim:
                        nc.sync.dma_start(recvb[:, :], sendb[:, :])
                    else:
                        nc.gpsimd.collective_compute(
                            "AllToAll", mybir.AluOpType.bypass,
                            replica_groups=[list(range(NCORES))],
                            ins=[sendb.opt()], outs=[recvb.opt()])

                # ---- phase D: output projection (t-sharded, full Wo), split by
                # head parity: even heads (recv1) accumulate + evict to SBUF
                # while A2A #2 is still in flight; odd heads finish and the DVE
                # sums the halves during eviction. ----
                ctx_e, ctx_o = [], []
                for c in range(NCORES):
                    t_ = ccp.tile([128, TS], dtb, tag="cc", name=f"cce{c}")
                    nc.sync.dma_start(t_[:], recv1[c * 128:(c + 1) * 128, :])
                    ctx_e.append(t_)
                po_list = []
                for oc in range(KC):
                    ps_o = psp.tile([128, TS], dtf, tag="mm")
                    for i, g in enumerate(range(0, KC, 2)):
                        wo_c = wo_s[:, g * D + oc * 128: g * D + (oc + 1) * 128]
                        nc.tensor.matmul(ps_o[:], wo_c, ctx_e[g // 2][:],
                                         start=(i == 0), stop=(i == KC // 2 - 1))
                    po = pop.tile([128, TS], dtb, tag="po", name=f"po{oc}")
                    nc.scalar.copy(po[:], ps_o[:])
                    po_list.append(po)
                for c in range(NCORES):
                    t_ = ccp.tile([128, TS], dtb, tag="cco", name=f"cco{c}")
                    nc.sync.dma_start(t_[:], recv2[c * 128:(c + 1) * 128, :])
                    ctx_o.append(t_)
                for oc in range(KC):
                    if oc < KC - 1:
                        ps_o = psp.tile([128, TS], dtf, tag="mm")
                        for i, g in enumerate(range(1, KC, 2)):
                            wo_c = wo_s[:, g * D + oc * 128: g * D + (oc + 1) * 128]
                            nc.tensor.matmul(ps_o[:], wo_c, ctx_o[g // 2][:],
                                             start=(i == 0), stop=(i == KC // 2 - 1))
                        ot = otp.tile([128, TS], dtf)
                        nc.vector.tensor_add(ot[:], ps_o[:], po_list[oc][:])
                        nc.sync.dma_start(outp[oc * 128:(oc + 1) * 128, :], ot[:])
                    else:
                        # last oc: two half-width groups so the first half's
                        # eviction overlaps the second half's matmuls
                        ot = otp.tile([128, TS], dtf)
                        for h in range(2):
                            cl, ch = h * (TS // 2), (h + 1) * (TS // 2)
                            ps_o = ps2p.tile([128, TS // 2], dtf, tag="acc")
                            for i, g in enumerate(range(1, KC, 2)):
                                wo_c = wo_s[:, g * D + oc * 128: g * D + (oc + 1) * 128]
                                nc.tensor.matmul(ps_o[:], wo_c,
                                                 ctx_o[g // 2][:, cl:ch],
                                                 start=(i == 0),
                                                 stop=(i == KC // 2 - 1))
                            nc.vector.tensor_add(ot[:, cl:ch], ps_o[:],
                                                 po_list[oc][:, cl:ch])
                            nc.sync.dma_start(
                                outp[oc * 128:(oc + 1) * 128, cl:ch],
                                ot[:, cl:ch])

            for _rep in range(reps):
                body()

    nc.compile()
    _cache[key] = nc
    return nc


def host_prep(x, Wq, Wk, Wv, Wo):
    bf16 = ml_dtypes.bfloat16
    x = np.asarray(x, dtype=np.float32)
    Wq = np.asarray(Wq, dtype=np.float32)
    Wk = np.asarray(Wk, dtype=np.float32)
    Wv = np.asarray(Wv, dtype=np.float32)
    Wo = np.asarray(Wo, dtype=np.float32)

    xT = np.ascontiguousarray(x.reshape(BT, D).T).astype(bf16)
    # Partition p (quadrant q = p//32, lane l = p%32) holds rope pair
    # d = q*16 + l%16; lanes 0-15 the even element (2d), 16-31 the odd
    # (2d+1) — so the rope partner sits 16 lanes away in-quadrant.
    p_ = np.arange(128)
    q_, l_ = p_ // 32, p_ % 32
    d_ = q_ * 16 + (l_ % 16)
    perm = 2 * d_ + (l_ >= 16)                                   # [128]

    pos = np.arange(T, dtype=np.float64)
    inv = ROPE_THETA ** (-np.arange(0, DH, 2, dtype=np.float64) / DH)  # [64]
    ang = inv[:, None] * pos[None, :]                                  # [64, T]
    c64 = np.cos(ang)
    s64 = np.sin(ang)
    cos128 = c64[d_].astype(bf16)                                # [128, T]
    sin128 = (np.where((l_ >= 16)[:, None], s64[d_], -s64[d_])).astype(bf16)

    tl = np.arange(128)[:, None]
    rl = np.arange(128)[None, :]
    msk = (tl <= rl).astype(np.float32).astype(bf16)             # [128, 128]

    ones = np.ones((128, 128), dtype=np.float32).astype(bf16)

    in_maps = []
    for i in range(NCORES):
        idx = np.concatenate([i * DC + h * DH + perm for h in range(HPC)])
        in_maps.append({
            "xT": xT,
            "wq": np.ascontiguousarray(Wq[:, idx]).astype(bf16),
            "wk": np.ascontiguousarray(Wk[:, idx]).astype(bf16),
            "wv": np.ascontiguousarray(Wv[:, i * DC:(i + 1) * DC]).astype(bf16),
            "wo": Wo.astype(bf16),
            "cosd": cos128, "sind": sin128, "mskd": msk,
            "onesd": ones,
        })
    return in_maps


def assemble(results):
    out_T = np.concatenate([results[i]["out"] for i in range(NCORES)], axis=1)
    return np.ascontiguousarray(out_T.T).reshape(B, T, D).astype(np.float32)


def kernel(x, Wq, Wk, Wv, Wo):
    nc = build()
    in_maps = host_prep(x, Wq, Wk, Wv, Wo)
    r = bass_utils.run_bass_kernel_spmd(nc, in_maps,
                                        core_ids=list(range(NCORES)))
    return assemble(r.results)

